# revision 56
# baseline (speedup 1.0000x reference)
"""Falcon-style MQA attention (71 heads, 1 KV head, RoPE, causal) on 8 TRN2 NeuronCores.

Sharding: 2D tensor-parallel — (batch, head-group) = (c // 4, c % 4); head
groups of [18, 18, 18, 17] query heads, the single KV head replicated per
batch-half (4x instead of 8x). Per core: bf16 QKV projection for its 18 head
slots (+KV) over its 1024 tokens, RoPE, causal flash-style attention in
transposed layout, then a PARTIAL dense projection over the core's own head
rows for all 4544 output columns. The host sums the 4 bf16 partial outputs per
batch in f32 (contraction-sharded dense = host reduce); no device collective.

Why 2D instead of 1D-over-heads: PE matmul cost is (row-chunks x k-tiles x
moving columns). 1D: 6 row-chunks x 2048 tokens; 2D: 10 row-chunks x 1024
tokens (1152 q rows + 128 kv = 1280, zero padding) — 17% less QKV work, and
the dense contraction is exactly 9x128 rows (vs 640-padded-from-576).

Performance notes (measured ~459us vs ~533us for the 1D-sharded baseline):
- Deep cross-phase pipelining: attention on the first 512 tokens overlaps the
  QKV matmuls of the second 512; dense (first 4 token tiles) overlaps the
  second attention half. Four attention heads in flight, admitted per-rc as
  RoPE lands, staggered so their Act-latency tails don't align.
- Everything the PE touches is bf16; accumulation stays f32.
- v_nat carries 64 all-ones columns, so the AV matmul emits the softmax
  denominator replicated on PSUM partitions 64-127 at no extra column cost;
  1/d = exp(-ln(d)) on Act, then one DVE multiply straight out of PSUM —
  no PE broadcast and no f32 staging copy. The act-table cache is nudged so
  Exp/Ln/Copy share one table set.
- hs/wq/wd arrive host-pretransposed in partition-major layout so every load
  is 128 large contiguous DMA descriptors. All wd slabs ride one DGE ring in
  ci order so the first 1.2MB slab lands ~2us after the WAR on the freed wq
  space resolves (three parallel rings would finish together ~16us later and
  head-of-line block the PE queue at the first dense matmul).

Self-contained: hardcodes all shapes; needs only numpy + ml_dtypes + concourse.
"""

import math
from contextlib import ExitStack

import numpy as np
import ml_dtypes

import concourse.mybir as mybir
import concourse.tile as tile
from concourse import bacc
from concourse.bass_utils import run_bass_kernel_spmd
from concourse.hw_specs import get_activation_tables


def _prioritize_act_table(arch):
    """Make the table-load pass resolve Exp/Ln/Copy to the single set that
    holds all three (avoiding a 1.3us table reload between every softmax Exp
    and denominator Ln). Dict ORDER must be preserved — the emitted
    act_func_set_id is positional — so instead of reordering we strip these
    functions from every other set in the cached dict."""
    tabs = get_activation_tables(arch)
    name = "natural_log_exp_and_others"
    if name not in tabs:
        return
    drop = {f for f in tabs[name] if f.name in ("Exp", "Ln", "Copy", "Identity")}
    for k, s in tabs.items():
        if k != name:
            s -= drop

NCORES = 8
N, L, D = 2, 1024, 4544
H, DKV = 71, 64
DP = 4608                    # D padded to 36*128
KT = DP // 128               # 36 contraction tiles for QKV
HPC = 18                     # head slots per core (group 3: 17 real + 1 pad)
QROWS = HPC * DKV            # 1152 attention rows per core (9*128 exact)
RROWS = QROWS + 2 * DKV      # 1280 fused rows per core (q + v + k)
RC = 10                      # row-chunks of fusedT (10 x 128 exact)
QKT = QROWS // 128           # 9 dense contraction tiles
MCH = 256                    # QKV token-chunk width
NCH = L // MCH               # 4 chunks per core
CI = 9                       # dense column blocks (8*512 + 448)
DCP = CI * 512               # 4608 padded dense cols
ROPE_BASE = 10000.0

F32 = mybir.dt.float32
BF16 = mybir.dt.bfloat16


def _build():
    nc = bacc.Bacc("TRN2", target_bir_lowering=False, debug=False, num_devices=NCORES)
    _prioritize_act_table(nc.m.arch)

    # all three weights/activations arrive pre-transposed to the exact SBUF
    # tile layout (partition-major) so every load is 128 large contiguous
    # descriptors instead of thousands of 256B ones
    hs_bf = nc.dram_tensor("hs_bf", [128, NCH, KT, MCH], BF16, kind="ExternalInput")
    wq_bf = nc.dram_tensor("wq_bf", [128, RC, KT, 128], BF16, kind="ExternalInput")
    wd_bf = nc.dram_tensor("wd_bf", [128, CI, QKT, 512], BF16, kind="ExternalInput")
    cos2 = nc.dram_tensor("cos2", [128, L], F32, kind="ExternalInput")
    sin2 = nc.dram_tensor("sin2", [128, L], F32, kind="ExternalInput")
    tri_in = nc.dram_tensor("tri", [128, 128], BF16, kind="ExternalInput")
    prope2 = nc.dram_tensor("prope2", [128, 128], BF16, kind="ExternalInput")
    ident64 = nc.dram_tensor("ident64", [64, 64], BF16, kind="ExternalInput")
    out = nc.dram_tensor("out", [L, D], BF16, kind="ExternalOutput")

    with tile.TileContext(nc) as tc, ExitStack() as top:
        constp = top.enter_context(tc.tile_pool(name="const", bufs=1))
        workp = top.enter_context(tc.tile_pool(name="work", bufs=2))
        recp = top.enter_context(tc.tile_pool(name="rec", bufs=2))
        # PSUM: 8 banks total = 2 (qkv accum / dense) + 2 (scores / rope /
        # v-transpose) + 4 (attention AV accumulators, one per in-flight head)
        psQ = top.enter_context(tc.tile_pool(name="psQ", bufs=2, space="PSUM"))
        psS = top.enter_context(tc.tile_pool(name="psS", bufs=2, space="PSUM"))
        psAV = top.enter_context(tc.tile_pool(name="psAV", bufs=4, space="PSUM"))

        # ---- constants (all DMA dispatch on the idle GpSimd sequencer) ----
        cosT = constp.tile([128, L], F32)
        sinT = constp.tile([128, L], F32)
        tri = constp.tile([128, 128], BF16)
        prope = constp.tile([128, 128], BF16)
        id64 = constp.tile([64, 64], BF16)

        fusedp = top.enter_context(tc.tile_pool(name="fused", bufs=1))
        fusedT = fusedp.tile([128, RC, L], BF16)

        attnp = top.enter_context(tc.tile_pool(name="attn", bufs=1))
        expp = top.enter_context(tc.tile_pool(name="exps", bufs=8))
        kT_dup = attnp.tile([128, L], BF16)
        # v in cols 0-63, all-ones in cols 64-127: the AV matmul then yields
        # the softmax denominator replicated on PSUM partitions 64-127 (same
        # column cost), so no PE broadcast of 1/denom is ever needed
        v_nat = attnp.tile([128, 8, 128], BF16)
        nc.vector.memset(v_nat[:, :, DKV:], 1.0)
        attn_sb = attnp.tile([128, QKT, L], BF16)

        stageA = ExitStack()
        wqp = stageA.enter_context(tc.tile_pool(name="wq", bufs=1))
        hstp = stageA.enter_context(tc.tile_pool(name="hst", bufs=2))

        wqT = wqp.tile([128, RC, KT, 128], BF16)
        nc.gpsimd.dma_start(wqT[:, 0, 0:1], wq_bf[:, 0, 0:1])
        nc.gpsimd.dma_start(wqT[:, 0, 1:3], wq_bf[:, 0, 1:3])
        nc.gpsimd.dma_start(wqT[:, 0, 3:8], wq_bf[:, 0, 3:8])
        nc.gpsimd.dma_start(wqT[:, 0, 8:18], wq_bf[:, 0, 8:18])
        nc.gpsimd.dma_start(wqT[:, 0, 18:KT], wq_bf[:, 0, 18:KT])
        for rc in range(1, RC):
            nc.gpsimd.dma_start(wqT[:, rc], wq_bf[:, rc])
        # constants are first needed ~60us in (RoPE onward) — dispatch them
        # behind the weight tiles so the first QKV matmuls start sooner
        nc.gpsimd.dma_start(cosT[:], cos2[:])
        nc.gpsimd.dma_start(sinT[:], sin2[:])
        nc.gpsimd.dma_start(tri[:], tri_in[:])
        nc.gpsimd.dma_start(prope[:], prope2[:])
        nc.gpsimd.dma_start(id64[:], ident64[:])

        stageB = ExitStack()          # opened mid-stream, after stageA closes

        def qkv_chunk(mc, after_rc=None, rc_order=None):
            """Emit QKV for one 256-token chunk; yields every 12 matmuls.
            after_rc: optional {rc: generator} emitted after that rc's copy."""
            hsT = hstp.tile([128, KT, MCH], BF16, tag="hsT")
            if mc == 0:
                # early pieces small for latency, later ones large so their
                # per-partition descriptors aren't starved by wq's 9KB ones
                # in the shared DMA queues' round-robin
                bnds = [0, 1, 2, 4, 8, 18, KT]
                engs = (nc.sync, nc.scalar)
                for i in range(len(bnds) - 1):
                    engs[i % 2].dma_start(hsT[:, bnds[i]:bnds[i + 1]],
                                          hs_bf[:, 0, bnds[i]:bnds[i + 1]])
            else:
                for q4 in range(4):
                    ksl = slice(KT // 4 * q4, KT // 4 * (q4 + 1))
                    eng = nc.scalar if q4 % 2 == 1 else nc.sync
                    eng.dma_start(hsT[:, ksl], hs_bf[:, mc, ksl])
            for rc in (rc_order or range(RC)):
                ps = psQ.tile([128, 512], F32, tag="q")
                for kt in range(KT):
                    nc.tensor.matmul(
                        ps[:, :MCH], wqT[:, rc, kt, :],
                        hsT[:, kt, :], start=(kt == 0), stop=(kt == KT - 1))
                    if kt % 12 == 11:
                        yield
                nc.vector.tensor_copy(
                    fusedT[:, rc, MCH * mc:MCH * (mc + 1)], ps[:, :MCH])
                if after_rc and rc in after_rc:
                    yield from after_rc[rc]

        def rope_unit(rc, hf, ksl_only=False):
            """RoPE in place on one [128, 512] block of q rows, or on the
            k-half (partitions 64-127) of the kv row-chunk."""
            sl = slice(512 * hf, 512 * (hf + 1))
            if ksl_only:
                x = fusedT[64:128, rc, :]
                pp = psS.tile([128, 512], F32, tag="s")
                nc.tensor.matmul(pp[64:128, :], prope[64:128, 64:128], x[:, sl],
                                 start=True, stop=True)
                a = workp.tile([128, 512], F32, tag="ropea")
                b = workp.tile([128, 512], F32, tag="ropeb")
                nc.gpsimd.tensor_mul(a[64:128, :], x[:, sl], cosT[64:128, sl])
                nc.vector.tensor_mul(b[64:128, :], pp[64:128, :], sinT[64:128, sl])
                nc.gpsimd.tensor_add(x[:, sl], a[64:128, :], b[64:128, :])
                return
            x = fusedT[:, rc, :]
            pp = psS.tile([128, 512], F32, tag="s")
            nc.tensor.matmul(pp[:], prope[:], x[:, sl], start=True, stop=True)
            a = workp.tile([128, 512], F32, tag="ropea")
            b = workp.tile([128, 512], F32, tag="ropeb")
            nc.gpsimd.tensor_mul(a[:], x[:, sl], cosT[:, sl])
            nc.vector.tensor_mul(b[:], pp[:], sinT[:, sl])
            nc.gpsimd.tensor_add(x[:, sl], a[:], b[:])

        def k_dup(hf):
            """Duplicate roped k rows into both partition halves so lhsT/rhs
            base partitions match for every head."""
            sl = slice(512 * hf, 512 * (hf + 1))
            src = fusedT[64:128, RC - 1, sl]
            nc.gpsimd.dma_start(kT_dup[0:64, sl], src)
            nc.gpsimd.dma_start(kT_dup[64:128, sl], src)
            yield

        def v_prep(hf):
            """Transpose v (partitions 0-63 of the kv row-chunk) into natural
            [keys, dkv] bf16 layout."""
            for jt in range(4 * hf, 4 * (hf + 1)):
                tp = psS.tile([128, 64], BF16, tag="s")
                nc.tensor.transpose(
                    tp[:],
                    fusedT[0:64, RC - 1, 128 * jt:128 * (jt + 1)],
                    id64[:])
                nc.vector.tensor_copy(v_nat[:, jt, 0:DKV], tp[:])
                yield

        def chain(*gens):
            for g in gens:
                yield from g

        def rope_gen(hf):
            for rc in range(RC - 1):
                rope_unit(rc, hf)
                yield

        def epilogue(hf):
            """RoPE + k/v prep hooks for the last QKV chunk of each 512-half.
            Keyed on the rc whose fusedT copy must land first. Each hook
            records readiness so attention heads can be admitted per-rc."""
            kv = RC - 1
            hooks = {kv: chain(_k_hook(hf), v_prep(hf), _kv_flag(hf))}
            for rc in range(RC - 1):
                hooks[rc] = _q_hook(rc, hf)
            return hooks

        def _k_hook(hf):
            rope_unit(RC - 1, hf, ksl_only=True)
            yield
            yield from k_dup(hf)

        def _kv_flag(hf):
            done["kv%d" % hf] = True
            return
            yield

        def _q_hook(rc, hf):
            rope_unit(rc, hf)
            done["rope%d" % hf].add(rc)
            yield

        def attn_head(h, qc):
            """Generator: one attention head's 512-query half, yielding
            between j-tile units."""
            poff = (64 * h) % 128
            prc = (64 * h) // 128
            kTn = kT_dup[poff:poff + 64, :]
            qh = fusedT[poff:poff + 64, prc, :]
            av = psAV.tile([128, 512], F32, tag="av")
            njt = 4 * (qc + 1)
            pend = None
            for jt in range(njt):
                off = max(0, 128 * jt - 512 * qc)
                sp = psS.tile([128, 512], F32, tag="s")
                nc.tensor.matmul(
                    sp[:, 0:512 - off],
                    kTn[:, 128 * jt:128 * (jt + 1)],
                    qh[:, 512 * qc + off:512 * (qc + 1)],
                    start=True, stop=True)
                et = expp.tile([128, 512], BF16, tag="exp")
                nc.scalar.activation(
                    et[:, off:512], sp[:, 0:512 - off],
                    mybir.ActivationFunctionType.Exp,
                    scale=1.0 / math.sqrt(DKV))
                if 128 * jt >= 512 * qc:
                    # diagonal-block causal mask on GpSimd: keeps the DVE
                    # queue (fusedT copies, attn muls) off the AV chain
                    nc.gpsimd.tensor_mul(
                        et[:, off:off + 128], et[:, off:off + 128], tri[:])
                if pend is not None:
                    pjt, po, pet = pend
                    nc.tensor.matmul(
                        av[:, po:512], v_nat[:, pjt, :], pet[:, po:512],
                        start=(pjt == 0), stop=False)
                pend = (jt, off, et)
                yield
            pjt, po, pet = pend
            nc.tensor.matmul(
                av[:, po:512], v_nat[:, pjt, :], pet[:, po:512],
                start=(pjt == 0), stop=True)
            # av rows 64-127 all hold the softmax denominator (ones columns of
            # v_nat); 1/d = exp(-ln(d)) on Act, then one fused DVE multiply
            # straight out of PSUM — no PE broadcast, no f32 staging copy
            lnd = recp.tile([64, 512], F32, tag="lnd")
            nc.scalar.activation(lnd[:], av[64:128, :],
                                 mybir.ActivationFunctionType.Ln)
            rec = recp.tile([64, 512], BF16, tag="rec")
            nc.scalar.activation(rec[:], lnd[:],
                                 mybir.ActivationFunctionType.Exp,
                                 scale=-1.0)
            yield
            nc.vector.tensor_mul(
                attn_sb[poff:poff + 64, prc, 512 * qc:512 * (qc + 1)],
                av[0:64, :], rec[:])
            yield

        wd_holder = []

        def dense_chunk(mt, ci, ot):
            wdT2 = wd_holder[0][ci]
            w = 448 if ci == CI - 1 else 512
            col = 512 * ci
            pa = psQ.tile([128, 512], F32, tag="q")
            for kt in range(QKT):
                nc.tensor.matmul(
                    pa[:, :w], attn_sb[:, kt, 128 * mt:128 * (mt + 1)],
                    wdT2[:, kt, 0:w],
                    start=(kt == 0), stop=(kt == QKT - 1))
            if ci % 3 != 2:
                nc.vector.tensor_copy(ot[:, col:col + w], pa[:, :w])
            else:
                nc.scalar.copy(ot[:, col:col + w], pa[:, :w])

        done = {"qc0": 0, "qc1": 0, "hf1": False,
                "kv0": False, "kv1": False,
                "rope0": set(), "rope1": set()}
        STORE_CI = {3: (0, 2048), 5: (2048, 3072), 7: (3072, 4096),
                    8: (4096, D)}

        wd_sent = set()

        def wd_fetch(ci):
            """All wd slabs ride ONE DGE ring in ci order: descriptors drain
            roughly in order through the shared queues, so ci0 completes
            ~2us after the WAR on the freed wq space resolves instead of
            after the whole 10.6MB load."""
            if ci < CI and ci not in wd_sent:
                wd_sent.add(ci)
                nc.gpsimd.dma_start(wd_holder[0][ci][:], wd_bf[:, ci])

        def dense_phase(mts, ots):
            """ci-major so column-block ci only waits on its own wd slab;
            staggered partial stores keep the output DMA off the tail."""
            for ci in range(CI):
                wd_fetch(ci + 2)
                for i, mt in enumerate(mts):
                    dense_chunk(mt, ci, ots[i])
                    yield
                    if ci in STORE_CI:
                        c0, c1 = STORE_CI[ci]
                        eng = nc.gpsimd if mt % 2 == 0 else nc.sync
                        eng.dma_start(out[128 * mt:128 * (mt + 1), c0:c1],
                                      ots[i][:, c0:c1])

        otp_holder = []

        def aux_gen():
            # QKV second half, then wd load, then dense over the first four
            # token tiles (overlapping second-half attention), then the rest
            for mc in range(2, NCH):
                yield from qkv_chunk(
                    mc, epilogue(1) if mc == NCH - 1 else None,
                    rc_order=[RC - 1] + list(range(RC - 1)) if mc == NCH - 1
                    else None)
            done["hf1"] = True
            stageA.close()
            wdp = stageB.enter_context(tc.tile_pool(name="wd", bufs=1))
            otp = stageB.enter_context(tc.tile_pool(name="ot", bufs=1))
            # one tile per ci column-block: dependency tracking then lets
            # dense ci start on its own 1.2MB slab instead of the full 10.6MB
            wds = [wdp.tile([128, QKT, 512], BF16, tag=f"wd{ci}",
                            name=f"wd{ci}") for ci in range(CI)]
            wd_holder.append(wds)
            wd_fetch(0)
            wd_fetch(1)
            ots = [otp.tile([128, D], BF16, tag=f"ot{i}", name=f"ot{i}")
                   for i in range(4)]
            otp_holder.append(ots)
            yield
            # hold dense back until enough qc1 attention is emitted ahead of
            # it in the PE queue to cover the wd DMA (in-order queue: a dense
            # matmul waiting on wd would head-of-line block later attention)
            while done["qc0"] < HPC or done["qc1"] < 10:
                yield
            yield from dense_phase([0, 1, 2, 3], ots)
            while done["qc1"] < HPC:
                yield
            yield from dense_phase([4, 5, 6, 7], ots)

        # ---- first half: QKV chunks 0-1 with RoPE / k/v prep interleaved
        # into chunk 1 ----
        for mc in range(2):
            for _ in qkv_chunk(mc, epilogue(0) if mc == 1 else None,
                               rc_order=[RC - 1] + list(range(RC - 1))
                               if mc == 1 else None):
                pass

        # ---- remaining phases: master round-robin scheduler ----
        heads_q = [(0, h) for h in range(HPC)] + [(1, h) for h in range(HPC)]
        active = []
        MAXH = 4
        aux = aux_gen()
        aux_done = False
        while active or heads_q or not aux_done:
            while len(active) < MAXH and heads_q:
                qc, h = heads_q[0]
                # admit a head as soon as its own row-chunk is roped and the
                # kv prep for its half is in — attention then starts inside
                # the tail of the QKV stream instead of 12us after it
                if not (done["kv%d" % qc] and (h // 2) in done["rope%d" % qc]):
                    break
                heads_q.pop(0)
                g = attn_head(h, qc)
                # stagger so concurrent heads don't hit their Act-latency
                # tails in lockstep
                for _ in range(3 * len(active)):
                    try:
                        next(g)
                    except StopIteration:
                        break
                active.append((qc, g))
            if not aux_done:
                try:
                    next(aux)
                except StopIteration:
                    aux_done = True
            for item in list(active):
                qc, g = item
                try:
                    next(g)
                except StopIteration:
                    active.remove(item)
                    done["qc%d" % qc] += 1
        stageB.close()

    nc.compile()
    return nc


_NC_CACHE = None


def _get_nc():
    global _NC_CACHE
    if _NC_CACHE is None:
        _NC_CACHE = _build()
    return _NC_CACHE


HGRP = [0, 18, 36, 54, 71]   # head-group boundaries


def _host_inputs(hidden_states, w_qkv, w_dense):
    """Build the per-core input maps (transpose + slice + bf16 cast on host)."""
    hs = np.asarray(hidden_states, dtype=np.float32)
    w_qkv = np.asarray(w_qkv, dtype=np.float32)
    w_dense = np.asarray(w_dense, dtype=np.float32)

    # per-batch hs: [DP, L] -> [128, L/MCH, KT, MCH] partition-major
    hs_maps = []
    for n in range(N):
        hs_t = np.zeros((DP, L), dtype=ml_dtypes.bfloat16)
        hs_t[:D, :] = np.ascontiguousarray(hs[n].T).astype(ml_dtypes.bfloat16)
        hs_maps.append(np.ascontiguousarray(
            hs_t.reshape(KT, 128, NCH, MCH).transpose(1, 2, 0, 3)))

    # RoPE tables, transposed to [dkv, l], duplicated on partitions 0-63 / 64-127
    inv_freq = 1.0 / (ROPE_BASE ** (np.arange(0, DKV, 2, dtype=np.float32) / DKV))
    t = np.arange(L, dtype=np.float32)
    freqs = np.outer(t, inv_freq)
    emb = np.concatenate([freqs, freqs], axis=-1)        # [L, DKV]
    cosT = np.cos(emb).T.astype(np.float32)              # [DKV, L]
    sinT = np.sin(emb).T.astype(np.float32)
    cos2 = np.concatenate([cosT, cosT], axis=0)          # [128, L]
    sin2 = np.concatenate([sinT, sinT], axis=0)

    # tri[j, q] = 1 if j <= q (within-tile causal mask)
    tri = (np.arange(128)[:, None] <= np.arange(128)[None, :]).astype(
        ml_dtypes.bfloat16)

    # RoPE rotation: (P x)[d] = -x[d+32] (d<32), x[d-32] (d>=32); lhsT = P.T, 2 blocks
    P1 = np.zeros((DKV, DKV), dtype=np.float32)
    for d in range(32):
        P1[d, d + 32] = -1.0
        P1[d + 32, d] = 1.0
    PT = P1.T
    prope2 = np.zeros((128, 128), dtype=ml_dtypes.bfloat16)
    prope2[:64, :64] = PT
    prope2[64:, 64:] = PT

    ident64 = np.eye(64, dtype=ml_dtypes.bfloat16)

    v_bf = w_qkv[(H + 1) * DKV:, :].T.astype(ml_dtypes.bfloat16)   # [D, 64]
    k_bf = w_qkv[H * DKV:(H + 1) * DKV, :].T.astype(ml_dtypes.bfloat16)
    wq_grp, wd_grp = [], []
    for g in range(4):
        h0, h1 = HGRP[g], HGRP[g + 1]
        nh = h1 - h0
        wq_loc = np.zeros((DP, RC * 128), dtype=ml_dtypes.bfloat16)
        wq_loc[:D, :nh * DKV] = w_qkv[h0 * DKV:h1 * DKV, :].T.astype(
            ml_dtypes.bfloat16)
        wq_loc[:D, QROWS:QROWS + DKV] = v_bf        # v on partitions 0-63
        wq_loc[:D, QROWS + DKV:RROWS] = k_bf        # k on partitions 64-127
        wq_grp.append(np.ascontiguousarray(
            wq_loc.reshape(KT, 128, RC, 128).transpose(1, 2, 0, 3)))

        # dense weight rows for this core's heads: w_dense columns
        # [64*h0 : 64*h1) transposed, zero-padded to QROWS rows and DCP cols,
        # ci-major so each 512-column block is one contiguous DMA slab
        wd_loc = np.zeros((QROWS, DCP), dtype=ml_dtypes.bfloat16)
        wd_loc[:nh * DKV, :D] = w_dense[:, DKV * h0:DKV * h1].T.astype(
            ml_dtypes.bfloat16)
        wd_grp.append(np.ascontiguousarray(
            wd_loc.reshape(QKT, 128, CI, 512).transpose(1, 2, 0, 3)))

    in_maps = []
    for c in range(NCORES):
        n, g = c // 4, c % 4
        in_maps.append({
            "hs_bf": hs_maps[n],
            "wq_bf": wq_grp[g],
            "wd_bf": wd_grp[g],
            "cos2": cos2,
            "sin2": sin2,
            "tri": tri,
            "prope2": prope2,
            "ident64": ident64,
        })
    return in_maps


def kernel(hidden_states, w_qkv, w_dense, _trace=False, _trace_kwargs=None):
    nc = _get_nc()
    in_maps = _host_inputs(hidden_states, w_qkv, w_dense)
    kw = {}
    if _trace:
        kw = dict(trace=True, **(_trace_kwargs or {}))
    res = run_bass_kernel_spmd(nc, in_maps, list(range(NCORES)), **kw)
    full = np.zeros((N, L, D), dtype=np.float32)
    for c in range(NCORES):
        full[c // 4] += res.results[c]["out"].astype(np.float32)
    kernel._last_exec_time_ns = res.exec_time_ns
    return full
